# revision 24
# baseline (speedup 1.0000x reference)
"""MoE sigmoid router (DeepSeek-style gate) on 8 TRN2 NeuronCores.

Data-parallel over tokens: each core scores 2048 tokens against 256 experts
(D=7168), applies grouped top-k masking (8 groups, keep 4 by top-2-sum),
then emits top-8 renormalized weights + expert indices.

The scoring matmul runs as a 3-term bf16 split (x = xh + xl, W = wh + wl;
logits = xh@wh + xh@wl + xl@wh) which keeps ~16 effective mantissa bits —
enough to reproduce the fp32 reference's top-k ordering — while running the
PE at bf16 rate (1 cycle/row vs 4 for fp32). x is transposed on the PE in
fp32 (4-chunk groups staged in PSUM) and split into bf16 hi/lo during the
PSUM->SBUF evacuation (Act cast + DVE subtract), so the split costs no extra
passes. The three bf16 chains all accumulate into one PSUM region; transpose
groups interleave with matmul groups so the PE never stalls.
"""

import numpy as np
from contextlib import ExitStack

import concourse.bass as bass
import concourse.tile as tile
from concourse import bacc, mybir
from concourse.bass_utils import run_bass_kernel_spmd
from concourse import masks

T, D, E = 16384, 7168, 256
G, TOPK_G, TOPK = 8, 4, 8
EPG = E // G          # 32 experts per group
ROUTE_SCALE = 2.5
NCORES = 8
TPC = T // NCORES     # 2048 tokens per core
P = 128
NT = TPC // P         # 16 token tiles per core
KC = D // P           # 56 contraction chunks
GRP = 4               # chunks per transpose-staging group (one PSUM bank)
NG = KC // GRP        # 14 groups per tile
F32 = mybir.dt.float32
BF16 = mybir.dt.bfloat16

_nc_cache = None


def build():
    nc = bacc.Bacc("TRN2", target_bir_lowering=False, debug=False)
    x_d = nc.dram_tensor("x", (TPC, D), F32, kind="ExternalInput")
    w_d = nc.dram_tensor("weight", (E, D), F32, kind="ExternalInput")
    b_d = nc.dram_tensor("bias", (1, E), F32, kind="ExternalInput")
    wout_d = nc.dram_tensor("wout", (TPC, TOPK), F32, kind="ExternalOutput")
    iout_d = nc.dram_tensor("iout", (TPC, TOPK), mybir.dt.uint32, kind="ExternalOutput")

    with tile.TileContext(nc) as tc, ExitStack() as ctx:
        const = ctx.enter_context(tc.tile_pool(name="const", bufs=1))
        xpool = ctx.enter_context(tc.tile_pool(name="xp", bufs=2))
        xhp = ctx.enter_context(tc.tile_pool(name="xhp", bufs=2))
        xlp = ctx.enter_context(tc.tile_pool(name="xlp", bufs=2))
        spool = ctx.enter_context(tc.tile_pool(name="sp", bufs=2))
        mpool = ctx.enter_context(tc.tile_pool(name="mp", bufs=2))
        tops = ctx.enter_context(tc.tile_pool(name="tops", bufs=2))
        opool = ctx.enter_context(tc.tile_pool(name="op", bufs=2))
        stg_psum = ctx.enter_context(tc.tile_pool(name="stg", bufs=6, space="PSUM"))
        acc_psum = ctx.enter_context(tc.tile_pool(name="acc", bufs=2, space="PSUM"))

        ident = const.tile([P, P], F32)
        masks.make_identity(nc, ident[:])
        bias_t = const.tile([1, E], F32)
        nc.sync.dma_start(bias_t[:], b_d[:, :])
        # bias broadcast across partitions; copied into PSUM before each tile's
        # accumulation (all matmuls run start=False), so no PE bias matmul.
        bias_bc = const.tile([P, E], F32)
        nc.gpsimd.partition_broadcast(bias_bc[:], bias_t[:])

        # --- weights: W [256, 7168] -> wcomb [128d, k, e|e'] bf16 where
        # [:, k, 0:256] = bf16(W^T) chunk k (hi) and [:, k, 256:512] = lo.
        # Loaded as four [128, D/2] quarters (d-half-major) so the first
        # matmul group only waits on the first two quarter loads.
        D2, KC2, NG2 = D // 2, KC // 2, NG // 2
        wcomb = const.tile([P, KC, 2 * E], BF16)
        wpool = ctx.enter_context(tc.tile_pool(name="wp", bufs=2))

        def w_quarter_dma(h, dh):
            # W rides the Act hwdge queue so it overlaps the x loads (SP queue)
            w_q = wpool.tile([P, D2], F32, name="w_sb")
            nc.scalar.dma_start(w_q[:], w_d[bass.ts(h, P), dh * D2 : (dh + 1) * D2])
            return w_q

        def w_prep_group(w_q, h, dh, g):
            stg = stg_psum.tile([P, GRP * P], F32, name="stg")
            for c in range(GRP):
                nc.tensor.transpose(
                    stg[:, bass.ts(c, P)],
                    w_q[:, bass.ts(GRP * g + c, P)],
                    ident[:],
                )
            stgv = stg[:].rearrange("p (k e) -> p k e", k=GRP)
            kg = dh * KC2 + GRP * g
            hi = wcomb[:, kg : kg + GRP, h * P : (h + 1) * P]
            lo = wcomb[:, kg : kg + GRP, E + h * P : E + (h + 1) * P]
            nc.scalar.copy(hi, stgv)
            nc.vector.tensor_sub(lo, stgv, hi)

        def t_group(x_t, xhT, xlT, g):
            stg = stg_psum.tile([P, GRP * P], F32, name="stg")
            for c in range(GRP):
                nc.tensor.transpose(
                    stg[:, bass.ts(c, P)],
                    x_t[:, bass.ts(GRP * g + c, P)],
                    ident[:],
                )
            stgv = stg[:].rearrange("p (k e) -> p k e", k=GRP)
            hi = xhT[:, GRP * g : GRP * (g + 1), :]
            lo = xlT[:, GRP * g : GRP * (g + 1), :]
            nc.scalar.copy(hi, stgv)
            nc.vector.tensor_sub(lo, stgv, hi)

        def m_group(acc, xhT, xlT, g):
            for c in range(GRP):
                k = GRP * g + c
                last = k == KC - 1
                nc.tensor.matmul(
                    acc[:], lhsT=xhT[:, k, :], rhs=wcomb[:, k, 0:E],
                    start=False, stop=False, skip_group_check=True,
                )
                nc.tensor.matmul(
                    acc[:], lhsT=xhT[:, k, :], rhs=wcomb[:, k, E : 2 * E],
                    start=False, stop=False, skip_group_check=True,
                )
                nc.tensor.matmul(
                    acc[:], lhsT=xlT[:, k, :], rhs=wcomb[:, k, 0:E],
                    start=False, stop=last, skip_group_check=True,
                )

        def postprocess(t, acc):
            s = spool.tile([P, E], F32)
            nc.scalar.activation(s[:], acc[:], mybir.ActivationFunctionType.Sigmoid)

            # group scores: top-2 sum within each group of 32
            gtop = tops.tile([P, G * 8], F32)
            for g in range(G):
                nc.vector.max(gtop[:, bass.ts(g, 8)], s[:, bass.ts(g, EPG)])
            gtv = gtop[:].rearrange("p (g k) -> p g k", g=G)
            gsum = tops.tile([P, G], F32)
            nc.vector.tensor_add(gsum[:], gtv[:, :, 0], gtv[:, :, 1])
            # top-4 groups -> per-group 0/1 mask via 4th-largest threshold
            gsort = tops.tile([P, 8], F32)
            nc.vector.max(gsort[:], gsum[:])
            gmask = tops.tile([P, G], F32)
            nc.vector.tensor_scalar(
                gmask[:], gsum[:], gsort[:, TOPK_G - 1 : TOPK_G], None,
                mybir.AluOpType.is_ge,
            )
            # masked scores (zeros outside kept groups; sigmoid > 0 everywhere)
            ms = mpool.tile([P, E], F32)
            for g in range(G):
                nc.gpsimd.tensor_scalar_mul(
                    ms[:, bass.ts(g, EPG)], s[:, bass.ts(g, EPG)], gmask[:, g : g + 1]
                )

            v8 = opool.tile([P, TOPK], F32)
            nc.vector.max(v8[:], ms[:])
            i8 = opool.tile([P, TOPK], mybir.dt.uint32)
            nc.vector.max_index(i8[:], v8[:], ms[:])

            t4 = opool.tile([P, 4], F32)
            nc.gpsimd.tensor_add(t4[:], v8[:, 0:4], v8[:, 4:8])
            t2 = opool.tile([P, 2], F32)
            nc.gpsimd.tensor_add(t2[:], t4[:, 0:2], t4[:, 2:4])
            sum1 = opool.tile([P, 1], F32)
            nc.gpsimd.tensor_add(sum1[:], t2[:, 0:1], t2[:, 1:2])
            rec = opool.tile([P, 1], F32)
            nc.vector.reciprocal(rec[:], sum1[:])
            w8 = opool.tile([P, TOPK], F32)
            nc.vector.tensor_scalar(
                w8[:], v8[:], rec[:], float(ROUTE_SCALE),
                mybir.AluOpType.mult, mybir.AluOpType.mult,
            )

            nc.sync.dma_start(wout_d[bass.ts(t, P), :], w8[:])
            nc.sync.dma_start(iout_d[bass.ts(t, P), :], i8[:])

        # --- emission. Tile 0 is special: its transposes keep the PE busy
        # while the four W quarter-loads land, and the first half of its
        # matmul groups runs as soon as the d0 quarters are prepped.
        x0 = xpool.tile([P, D], F32, name="x_t")
        nc.sync.dma_start(x0[:, 0:D2], x_d[bass.ts(0, P), 0:D2])
        wq01 = [w_quarter_dma(0, 0), w_quarter_dma(1, 0)]
        nc.sync.dma_start(x0[:, D2:D], x_d[bass.ts(0, P), D2:D])
        wq23 = [w_quarter_dma(0, 1), w_quarter_dma(1, 1)]

        xhT0 = xhp.tile([P, KC, P], BF16, name="xhT")
        xlT0 = xlp.tile([P, KC, P], BF16, name="xlT")
        acc0 = acc_psum.tile([P, E], F32, name="acc")
        nc.scalar.copy(acc0[:], bias_bc[:])
        for g in range(NG2):
            t_group(x0, xhT0, xlT0, g)
        for g in range(NG2):
            w_prep_group(wq01[0], 0, 0, g)
        for g in range(NG2):
            w_prep_group(wq01[1], 1, 0, g)
            m_group(acc0, xhT0, xlT0, g)
        for g in range(NG2, NG):
            t_group(x0, xhT0, xlT0, g)
        for g in range(NG2):
            w_prep_group(wq23[0], 0, 1, g)
        for g in range(NG2):
            w_prep_group(wq23[1], 1, 1, g)
            m_group(acc0, xhT0, xlT0, NG2 + g)
        postprocess(0, acc0)

        for t in range(1, NT):
            x_t = xpool.tile([P, D], F32, name="x_t")
            nc.sync.dma_start(x_t[:], x_d[bass.ts(t, P), :])
            xhT = xhp.tile([P, KC, P], BF16, name="xhT")
            xlT = xlp.tile([P, KC, P], BF16, name="xlT")
            acc = acc_psum.tile([P, E], F32, name="acc")
            nc.scalar.copy(acc[:], bias_bc[:])
            # interleave: 2 transpose groups of lookahead before each matmul group
            t_group(x_t, xhT, xlT, 0)
            t_group(x_t, xhT, xlT, 1)
            for g in range(NG):
                if g + 2 < NG:
                    t_group(x_t, xhT, xlT, g + 2)
                m_group(acc, xhT, xlT, g)
            postprocess(t, acc)

    nc.finalize()
    return nc


def _in_maps(x, weight, bias):
    x = np.ascontiguousarray(x, dtype=np.float32)
    weight = np.ascontiguousarray(weight, dtype=np.float32)
    bias2 = np.ascontiguousarray(bias, dtype=np.float32).reshape(1, E)
    return [
        {"x": x[i * TPC : (i + 1) * TPC], "weight": weight, "bias": bias2}
        for i in range(NCORES)
    ]


def kernel(x, weight, bias):
    global _nc_cache
    if _nc_cache is None:
        _nc_cache = build()
    res = run_bass_kernel_spmd(
        _nc_cache, _in_maps(x, weight, bias), core_ids=list(range(NCORES))
    )
    weights = np.concatenate([res.results[i]["wout"] for i in range(NCORES)], axis=0)
    indices = np.concatenate([res.results[i]["iout"] for i in range(NCORES)], axis=0)
    return weights.astype(np.float32), indices.astype(np.int32)


def run_profiled(inputs, trace_cores=None):
    """Re-run with NTFF tracing; returns BassKernelResults with exec_time_ns."""
    global _nc_cache
    if _nc_cache is None:
        _nc_cache = build()
    return run_bass_kernel_spmd(
        _nc_cache,
        _in_maps(**inputs),
        core_ids=list(range(NCORES)),
        trace=True,
        trace_cores=trace_cores,
    )
